# revision 32
# baseline (speedup 1.0000x reference)
"""Trainium2 Bass kernel: low-rank (LoRA-style) linear with 2:4 soft-threshold
pruned weights, fp16 matmul / fp32 accumulate.

  wA = soft_threshold24(weight_A) * scale_A          # [IN, R]
  wB = soft_threshold24(weight_B) * scale_B          # [OUT, R]
  x_proj = f16(x) @ f16(wA)            (f32 accum)   # [N, R]
  out    = f16(x_proj) @ f16(wB).T + bias            # [N, OUT]

Sharding: data-parallel over the token dim across 8 cores (2048 tokens/core),
small weights replicated. No collectives.

I/O: the reference computes both matmuls on f16 operands, so the host casts
x / weights / bias to f16 before upload and upcasts the f16 result to f32
after download. This halves HBM traffic and matches reference numerics.

Transpose trick (mm1 needs x with IN on partitions): bitcast pairs of
adjacent-i f16 values as one f32 and PE-transpose in f32 (bit-exact,
verified on adversarial patterns; 2 cyc/row for 2 f16 = same rate as f16
but HALF the instructions and half the PSUM->SBUF copy elements). The transposed layout interleaves the
even/odd feature sub-lanes along the free dim; mm1 contracts them with two
stride-2 f16 moving APs per pair-chunk, against a wA tile loaded in the
matching interleaved row order (the 2:4 threshold is elementwise along R,
so the row permutation doesn't affect it).

Per 512-token group: 64 f32 pair transposes -> ACT/DVE copies -> 32 mm1
matmuls -> xpa f16 cast (+ones row for the bias trick) -> 32 mm2 matmuls
vs wbt -> ACT/DVE copies -> f16 row store. Group-major so the PE streams
continuously from ~3us (p-state ramps) and stores spread over the whole
kernel. x loads ride the SP DMA queue, stores + weights the ACT queue.
"""

import sys

import numpy as np

if "/opt/trn_rl_repo" not in sys.path:
    sys.path.insert(0, "/opt/trn_rl_repo")

B, S, IN_F, OUT_F, RANK = 4, 4096, 4096, 4096, 64
N_CORES = 8
N_TOK = B * S                   # 16384
T_CORE = N_TOK // N_CORES       # 2048 tokens per core
P = 128
GTOK = 512                      # tokens per group
N_GRP = T_CORE // GTOK          # 4 groups per core
TPG = GTOK // P                 # 4 token tiles per group
N_KC = IN_F // (2 * P)          # 16 f32-pair chunks of 128
MM2_N = 512
N_OB = OUT_F // MM2_N           # 8 output column groups

_CACHE = {}


def _soft_threshold_weights(nc, pool, w_dram, scale, out_f16):
    """Emit IR computing soft_threshold24(w_dram f16) * scale (f16).

    w_dram: [IN_or_OUT, RANK] f16, already host-permuted so that partition
    p's nb rows are contiguous (see make_in_maps): a plain 4KB-descriptor
    load lands row-permuted weights as [p, n, r]. The 2:4 threshold is
    elementwise over groups of 4 along R, so any row permutation works.
    """
    import concourse.mybir as mybir

    f16 = mybir.dt.float16
    nb = w_dram.shape[0] // P
    wf = pool.tile([P, nb, RANK], f16, tag="wstage", name="wstage")
    nc.scalar.dma_start(wf[:], w_dram[:].rearrange("(c n) r -> c n r", c=P))

    thr = pool.tile([P, nb, RANK], f16, tag="wthr", name="wthr")
    amin = mybir.AluOpType.min
    amx = mybir.AluOpType.max
    ve = nc.vector

    wfh = wf[:]
    g = wfh.rearrange("p b (g q) -> p b g q", q=4)
    gj = [g[:, :, :, j : j + 1] for j in range(4)]
    ash = [P, nb, RANK // 4, 1]
    wneg = pool.tile([P, nb, RANK], f16, tag="wneg", name="wneg")
    ve.tensor_scalar_mul(wneg[:], wfh, -1.0)
    ng = wneg[:].rearrange("p b (g q) -> p b g q", q=4)
    ab = [pool.tile(ash, f16, tag=f"abs{j}", name=f"abs{j}") for j in range(4)]
    for j in range(4):
        ve.tensor_tensor(ab[j][:], gj[j], ng[:, :, :, j : j + 1], op=amx)
    m1 = pool.tile(ash, f16, tag="m1", name="m1")
    M1 = pool.tile(ash, f16, tag="M1", name="M1")
    m2 = pool.tile(ash, f16, tag="abs0", name="m2")
    M2 = pool.tile(ash, f16, tag="abs1", name="M2")
    ve.tensor_tensor(m1[:], ab[0][:], ab[1][:], op=amin)
    ve.tensor_tensor(M1[:], ab[0][:], ab[1][:], op=amx)
    ve.tensor_tensor(m2[:], ab[2][:], ab[3][:], op=amin)
    ve.tensor_tensor(M2[:], ab[2][:], ab[3][:], op=amx)
    # 2nd smallest of the 4 = min(max(m1, m2), min(M1, M2))
    t = pool.tile(ash, f16, tag="abs2", name="t")
    ve.tensor_tensor(m1[:], m1[:], m2[:], op=amx)
    ve.tensor_tensor(M1[:], M1[:], M2[:], op=amin)
    ve.tensor_tensor(t[:], m1[:], M1[:], op=amin)
    # t4: threshold broadcast over the group-of-4 axis
    t4 = pool.tile([P, nb, RANK], f16, tag="t4", name="t4")
    h4 = t4[:].rearrange("p b (g q) -> p b g q", q=4)
    for j in range(4):
        ve.tensor_copy(h4[:, :, :, j : j + 1], t[:])
    # s = w - clip(w, -t, t)
    th = thr[:]
    nt4 = pool.tile([P, nb, RANK], f16, tag="wneg", name="nt4")
    ve.tensor_scalar_mul(nt4[:], t4[:], -1.0)
    ve.tensor_tensor(th, wfh, t4[:], op=amin)
    ve.tensor_tensor(th, th, nt4[:], op=amx)
    ve.tensor_sub(th, wfh, th)
    if scale != 1.0:
        ve.tensor_scalar_mul(th, th, float(scale))
    if out_f16 is not None:
        ve.tensor_copy(out_f16[:], thr[:])
    return thr


def _build(scale_a, scale_b):
    import concourse.mybir as mybir
    import concourse.tile as tile
    from concourse import bacc
    from concourse.bass import ts
    from concourse.masks import make_identity

    f32, f16 = mybir.dt.float32, mybir.dt.float16
    f32r = mybir.dt.float32r

    nc = bacc.Bacc("TRN2", target_bir_lowering=False, debug=False,
                   enable_asserts=False)
    x_d = nc.dram_tensor("x", [T_CORE, IN_F], f16, kind="ExternalInput")
    wa_d = nc.dram_tensor("weight_A", [IN_F, RANK], f16, kind="ExternalInput")
    wb_d = nc.dram_tensor("weight_B", [OUT_F, RANK], f16, kind="ExternalInput")
    b_d = nc.dram_tensor("bias", [1, OUT_F], f16, kind="ExternalInput")
    o_d = nc.dram_tensor("out", [T_CORE, OUT_F], f16, kind="ExternalOutput")

    with tile.TileContext(nc) as tc:
        with (
            tc.tile_pool(name="const", bufs=1) as constp,
            tc.tile_pool(name="wtmp", bufs=1) as wtmp,
            tc.tile_pool(name="bulk", bufs=8) as bulkp,
            tc.tile_pool(name="xtp", bufs=2) as xtp,
            tc.tile_pool(name="outp", bufs=5) as outp,
            tc.tile_pool(name="proj", bufs=2) as projp,
            tc.tile_pool(name="ps1", bufs=1, space="PSUM") as ps1p,
            tc.tile_pool(name="pst", bufs=2, space="PSUM") as pstp,
            tc.tile_pool(name="ps2", bufs=3, space="PSUM") as ps2p,
        ):
            ident16 = constp.tile([P, P], f16)
            make_identity(nc, ident16[:])
            ident32 = constp.tile([P, P], f32)
            make_identity(nc, ident32[:])

            # --- x loads on the SP queue: full f16 rows per token tile ---
            bulks = []
            for i in range(T_CORE // P):
                bt = bulkp.tile([P, IN_F], f16, name="bulk", tag="bulk")
                nc.sync.dma_start(bt[:], x_d[ts(i, P), :])
                bulks.append(bt)

            # --- weights on the ACT queue; wA in interleaved row order:
            # wa16[c, 2k+s, r] = wA[(k*128+c)*2+s, r]  (pair-transpose layout)
            wa16 = constp.tile([P, 2 * N_KC, RANK], f16)
            _soft_threshold_weights(nc, wtmp, wa_d, scale_a, wa16)

            wbt = constp.tile([RANK + 1, OUT_F], f16)  # wB.T (+ bias row)
            thr_b = _soft_threshold_weights(nc, wtmp, wb_d, scale_b, None)
            nc.scalar.dma_start(wbt[RANK : RANK + 1, :], b_d[:])

            xts = [None] * N_GRP

            def transpose_quanta(g):
                """Yield PE-work quanta for group g's pair transposes.

                Each quantum is one f32 pair transpose; after every 8 (two
                k-chunks -> one 2-bank PSUM tile) the drain copy is emitted
                on an alternating engine. xt32[c, k, tt*128+dt] (f32) holds
                the (i=2(k*128+c), i+1) f16 pair of token g*512+tt*128+dt.
                """
                xt32 = xtp.tile([P, N_KC, TPG * P], f32, tag="xt", name="xt")
                xts[g] = xt32
                # token-tile-major: the first PSUM tile needs only bulks[0]
                HK = N_KC // 2
                for tt in range(TPG):
                    i = g * TPG + tt
                    for kh in range(2):
                        pt = pstp.tile([P, HK, P], f32, tag="ptx", name="pt")
                        for kx in range(HK):
                            k = kh * HK + kx
                            nc.tensor.transpose(
                                pt[:, kx, :],
                                bulks[i][:].bitcast(f32)[:, ts(k, P)],
                                ident32[:])
                            yield
                        dst = xt32[:, kh * HK : (kh + 1) * HK, ts(tt, P)]
                        # group 0: DVE is busy with weight prep - keep its
                        # consumers off the DVE queue
                        if g == 0 or kh == 1:
                            nc.scalar.copy(dst, pt[:])
                        else:
                            nc.vector.tensor_copy(dst, pt[:])

            def drain(it, n=None):
                if it is None:
                    return None
                try:
                    if n is None:
                        while True:
                            next(it)
                    else:
                        for _ in range(n):
                            next(it)
                except StopIteration:
                    return None
                return it

            # prologue: group 0 transposes
            drain(transpose_quanta(0))

            def wbt_quanta():
                """wbt transposes, interleaved into mm1 of group 0."""
                for b in range(OUT_F // P):
                    # PSUM slots borrowed from ps2 (mm2 waits on wbt anyway)
                    pw = ps2p.tile([P, MM2_N], f32, tag="ps2", name="pw")
                    pwv = pw[0:RANK, 0 : P // 2].bitcast(f16)
                    nc.tensor.transpose(pwv, thr_b[:, b, :], ident16[:])
                    if b % 2 == 0:
                        nc.scalar.copy(wbt[0:RANK, ts(b, P)], pwv)
                    else:
                        nc.vector.tensor_copy(wbt[0:RANK, ts(b, P)], pwv)
                    yield

            wq = wbt_quanta()

            for g in range(N_GRP):
                # mm1: two stride-2 f16 sub-lane matmuls per pair chunk
                acc = ps1p.tile([RANK, GTOK], f32, tag="acc", name="acc")
                xt16 = xts[g][:].bitcast(f16).rearrange(
                    "p k (t s) -> p k t s", s=2)
                for k in range(N_KC):
                    for sl in range(2):
                        nc.tensor.matmul(acc[:], wa16[:, 2 * k + sl, :],
                                         xt16[:, k, :, sl],
                                         start=(k == 0 and sl == 0),
                                         stop=(k == N_KC - 1 and sl == 1))
                        wq = drain(wq, 1)
                wq = drain(wq)

                xpa = projp.tile([RANK + 1, GTOK], f16)
                nc.scalar.copy(xpa[0:RANK, :], acc[:])
                nc.vector.memset(xpa[RANK : RANK + 1, :], 1.0)

                # mm2 of group g, interleaved with group g+1's transposes so
                # the PE never idles while PSUM drains (keeps p-state high)
                tq = transpose_quanta(g + 1) if g + 1 < N_GRP else None
                # fill the xpa round-trip latency (PSUM->ACT->SBUF->ldweights)
                tq = drain(tq, 10)
                for tt in range(TPG):
                    i = g * TPG + tt
                    ob = outp.tile([P, OUT_F], f16, name="ob", tag="ob")
                    for j in range(N_OB):
                        ps2 = ps2p.tile([P, MM2_N], f32, tag="ps2", name="ps2")
                        nc.tensor.matmul(ps2[:], xpa[:, ts(tt, P)],
                                         wbt[:, ts(j, MM2_N)],
                                         start=True, stop=True)
                        dst = ob[:, ts(j, MM2_N)]
                        # last group: ACT also issues the final stores, so
                        # lean on DVE for the drain copies
                        dve_js = (0, 2, 4, 6) if g == N_GRP - 1 else (0, 3, 6)
                        if j in dve_js:
                            nc.vector.tensor_copy(dst, ps2[:])
                        else:
                            nc.scalar.copy(dst, ps2[:])
                        # front-load the quanta so the last transpose
                        # copies land well before the next group's mm1
                        tq = drain(tq, 3 if tt < 2 else 1)
                    # late groups: the load queue (SP) has drained - split
                    # stores across both DMA queues
                    if g >= 2 and tt % 2 == 1:
                        nc.sync.dma_start(o_d[ts(i, P), :], ob[:])
                    else:
                        nc.scalar.dma_start(o_d[ts(i, P), :], ob[:])
                drain(tq)

    nc.compile()
    return nc


def get_nc(scale_a, scale_b):
    key = (float(scale_a), float(scale_b))
    if key not in _CACHE:
        _CACHE[key] = _build(*key)
    return _CACHE[key]


def make_in_maps(x, weight_A, weight_B, bias):
    """Host-side shard + f16 cast: per-core input dicts."""
    x16 = np.ascontiguousarray(np.asarray(x, dtype=np.float32).astype(np.float16))
    wa = np.asarray(weight_A, np.float32).astype(np.float16)
    wb = np.asarray(weight_B, np.float32).astype(np.float16)
    # Lossless row permutations so the device DMA is contiguous (4KB
    # descriptors instead of 128B row gathers):
    #   wa16[c, 2k+s, r] = wA[(k*128+c)*2+s, r] -> send rows in (c,k,s) order
    #   thr_b[p, b, r]   = wB[b*128+p, r]       -> send rows in (p,b) order
    wa = np.ascontiguousarray(
        wa.reshape(IN_F // 256, P, 2, RANK).transpose(1, 0, 2, 3)
        .reshape(IN_F, RANK))
    wb = np.ascontiguousarray(
        wb.reshape(OUT_F // P, P, RANK).transpose(1, 0, 2)
        .reshape(OUT_F, RANK))
    bi = np.ascontiguousarray(
        np.asarray(bias, np.float32).astype(np.float16)).reshape(1, OUT_F)
    xf = x16.reshape(N_TOK, IN_F)
    return [
        {
            "x": xf[c * T_CORE : (c + 1) * T_CORE],
            "weight_A": wa,
            "weight_B": wb,
            "bias": bi,
        }
        for c in range(N_CORES)
    ]


def kernel(x, weight_A, weight_B, bias, scale_A, scale_B):
    from concourse.bass_utils import run_bass_kernel_spmd

    sa = float(np.asarray(scale_A))
    sb = float(np.asarray(scale_B))
    nc = get_nc(sa, sb)

    in_maps = make_in_maps(x, weight_A, weight_B, bias)
    res = run_bass_kernel_spmd(nc, in_maps, core_ids=list(range(N_CORES)))
    out = np.concatenate([r["out"] for r in res.results], axis=0)
    return out.astype(np.float32).reshape(B, S, OUT_F)
